# revision 5
# baseline (speedup 1.0000x reference)
"""Trainium2 Bass kernel for nn_GaussianBlur: depthwise 2D conv, 71x71 kernel,
x [16,3,512,512] fp32.

Strategy (SVD-factorized operator, all-bf16, DMA-streaming pipeline):
  - The 2D kernel is separable (rank-1 outer product for a Gaussian); the 1D
    conv along each axis is a banded 512x512 Toeplitz operator T.
  - T is numerically low-rank (sigma=10 Gaussian): the rank-128
    factorization T = L @ R^T is exact to ~3e-4. The blur becomes four thin
    matmul passes per slice:
        A: M1t = X^T  @ Rx      (16 MM, N=128)
        B: M2  = M1t^T @ Ry     ( 4 MM, N=128)   M2 = Rx^T X Ry
        C: M3  = M2^T @ Lx^T    ( 1 MM, N=512)   M3 = (Lx M2)^T
        D: Y   = M3^T @ Ly^T    ( 4 MM, N=512)   Y  = Lx M2 Ly^T
  - All bf16 (fp32 PSUM accumulation): halves DMA vs fp32, 1 cycle/row PE,
    FWL weight loads. End-to-end rel err ~5e-3 (gate 2e-2).
  - x is relayouted host-side to partition-major [128, s, th, j] so ALL
    slice loads are issued up front (slice 0 split in halves for a fast
    start, interleaved with the constants on the scalar HWDGE queue; slices
    1..5 on the sync HWDGE queue). The SDMA engines then stream x
    continuously at line rate with no per-phase issue dependencies.
  - y stores are issued on the sync queue as each slice finishes; loads
    and stores never block each other's issue (separate ring positions,
    data-dependency waits only on the store side).
  - D-pass matmuls write into 2-bank PSUM tiles so each y slice needs only
    two [128,1024] PSUM->SBUF cast copies (vector + gpsimd) instead of four
    [128,512] ones; m1 copy on vector, m2/m3 on scalar.
  - Software pipeline, skew 3: phase p runs D(p-3), A(p), B(p-1), C(p-2)
    (D first so the store chain starts earliest). Final two slices' C/D
    share the last phases so the tail drains early.
  - PE warm-up matmuls on zeroed scratch fill the pre-data window so the
    HAM clock-gate ramp starts before the real matmuls.
  - For a symmetric kernel (kx == ky, the Gaussian case) Rx==Ry and
    Lx==Ly: shared constant tensors halve constant DMA traffic.
  - Data parallel: 48 (n,c) slices sharded 6-per-core across 8 NeuronCores.
"""

import sys

sys.path.insert(0, "/opt/trn_rl_repo")

from contextlib import ExitStack

import ml_dtypes
import numpy as np

import concourse.bass as bass
import concourse.tile as tile
from concourse import bacc, mybir
from concourse.bass import ts
from concourse.bass_utils import run_bass_kernel_spmd

N_CORES = 8
H = W = 512
PT = 128          # partition tile
NT = H // PT      # 4 tiles per 512 dim
SLICES = 6        # 16*3 / 8 per core
PAD = 35
KS = 71
R = 128           # factorization rank (= partition width)

BF16 = ml_dtypes.bfloat16

_kernel_cache = {}


def _build_bass(ncomp: int, sym: bool):
    """Per-core Bass module: ncomp separable components, rank-R factors.
    sym=True shares Rx/Ry and Lx/Ly tensors (symmetric kernel)."""
    f32 = mybir.dt.float32
    bf16 = mybir.dt.bfloat16

    nc = bacc.Bacc(name="gaussblur_svd")
    # x/y layout: partition-major [p, s, th, j] with x[s, 128*th + p, j] —
    # per-partition contiguous 4KB per slice; any slice range is one DMA.
    x_d = nc.dram_tensor("x", [PT, SLICES, NT, W], bf16, kind="ExternalInput")
    rx_d = nc.dram_tensor("rx", [PT, ncomp, NT, R], bf16, kind="ExternalInput")
    lxt_d = nc.dram_tensor("lxt", [PT, ncomp, H], bf16, kind="ExternalInput")
    if not sym:
        ry_d = nc.dram_tensor("ry", [PT, ncomp, NT, R], bf16, kind="ExternalInput")
        lyt_d = nc.dram_tensor("lyt", [PT, ncomp, W], bf16, kind="ExternalInput")
    y_d = nc.dram_tensor("y", [PT, SLICES, NT, W], bf16, kind="ExternalOutput")

    with tile.TileContext(nc) as tc, ExitStack() as ctx:
        const_pool = ctx.enter_context(tc.tile_pool(name="const", bufs=1))
        m1_pool = ctx.enter_context(tc.tile_pool(name="m1p", bufs=3))
        m2_pool = ctx.enter_context(tc.tile_pool(name="m2p", bufs=3))
        m3_pool = ctx.enter_context(tc.tile_pool(name="m3p", bufs=3))
        y_pool = ctx.enter_context(tc.tile_pool(name="yp", bufs=3))
        psa = ctx.enter_context(tc.tile_pool(name="psa", bufs=2, space="PSUM"))
        psb = ctx.enter_context(tc.tile_pool(name="psb", bufs=1, space="PSUM"))
        psc = ctx.enter_context(tc.tile_pool(name="psc", bufs=1, space="PSUM"))
        psd = ctx.enter_context(tc.tile_pool(name="psd", bufs=2, space="PSUM"))

        # PE warm-up: the HAM clock-gate grants full rate only after a few
        # us of sustained PE activity. Fill the pre-data window (from the
        # startup barrier until slice 0 lands) with matmuls on zeroed
        # scratch so the ramp is underway when the real work starts.
        warm_in = const_pool.tile([PT, W], bf16)
        nc.gpsimd.memset(warm_in[:], 0.0)
        for _ in range(5):
            ow = psa.tile([PT, NT * R], f32, name="oa", tag="oa")
            nc.tensor.matmul(ow[:], warm_in[:, 0:PT], warm_in[:],
                             start=True, stop=True)

        rx_t = const_pool.tile([PT, ncomp, NT, R], bf16)
        lxt_t = const_pool.tile([PT, ncomp, H], bf16)
        if sym:
            ry_t, lyt_t = rx_t, lxt_t
        else:
            ry_t = const_pool.tile([PT, ncomp, NT, R], bf16)
            lyt_t = const_pool.tile([PT, ncomp, W], bf16)

        # All input DMAs issued up front. Scalar queue: constants + slice-0
        # halves in consumption order (rx first so A(0) can start on the
        # first x half). Sync queue: slices 1..5, then the y stores later.
        xb = const_pool.tile([PT, SLICES, NT, W], bf16)
        nc.scalar.dma_start(rx_t[:], rx_d.ap()[:])
        nc.scalar.dma_start(xb[:, 0, 0:2, :], x_d.ap()[:, 0, 0:2, :])
        nc.scalar.dma_start(lxt_t[:], lxt_d.ap()[:])
        if not sym:
            nc.scalar.dma_start(ry_t[:], ry_d.ap()[:])
            nc.scalar.dma_start(lyt_t[:], lyt_d.ap()[:])
        nc.scalar.dma_start(xb[:, 0, 2:4, :], x_d.ap()[:, 0, 2:4, :])
        for s in range(1, SLICES):
            nc.sync.dma_start(xb[:, s], x_d.ap()[:, s])

        m1_tiles = {}
        m2_tiles = {}
        m3_tiles = {}

        NPH = SLICES + 2  # final slice's C and D share the last phase
        for p in range(NPH):
            # ---- D: Y = M3^T @ Ly^T ----
            # Emitted first in the phase so the y store chain starts as
            # early as possible. Normally slice p-3; the last two slices
            # are pulled forward (right after their C, below) so the final
            # output DMAs don't pile up into one tail burst.
            def emit_d(s):
                m3 = m3_tiles[s]
                y_t = y_pool.tile([PT, NT, W], bf16, name=f"y{s}", tag="y")
                for h in range(2):
                    od = psd.tile([PT, 2, W], f32, name="od", tag="od")
                    for j in range(2):
                        ti = 2 * h + j
                        for c in range(ncomp):
                            nc.tensor.matmul(
                                od[:, j, :],
                                m3[:, c, ts(ti, PT)],
                                lyt_t[:, c, :],
                                start=(c == 0),
                                stop=(c == ncomp - 1),
                            )
                    if h == 0:
                        nc.vector.tensor_copy(y_t[:, 0:2, :], od[:])
                    else:
                        nc.scalar.copy(y_t[:, 2:4, :], od[:])
                nc.sync.dma_start(y_d.ap()[:, s], y_t[:])

            if 0 <= p - 3 < SLICES - 2:
                emit_d(p - 3)

            # ---- A(p): M1t = X^T @ Rx ----
            if p < SLICES:
                m1t = m1_pool.tile([PT, ncomp, NT, R], bf16, name=f"m1_{p}",
                                   tag="m1")
                m1_tiles[p] = m1t
                for c in range(ncomp):
                    oa = psa.tile([PT, NT * R], f32, name="oa", tag="oa")
                    for th in range(NT):
                        for tw in range(NT):
                            nc.tensor.matmul(
                                oa[:, ts(tw, R)],
                                xb[:, p, th, ts(tw, PT)],
                                rx_t[:, c, th, :],
                                start=(th == 0 and tw == 0),
                                stop=(th == NT - 1 and tw == NT - 1),
                            )
                    nc.vector.tensor_copy(m1t[:, c], oa[:])

            # ---- B(p-1): M2 = M1t^T @ Ry ----
            if 0 <= p - 1 < SLICES:
                s = p - 1
                m1t = m1_tiles[s]
                m2 = m2_pool.tile([PT, ncomp, R], bf16, name=f"m2_{s}", tag="m2")
                m2_tiles[s] = m2
                for c in range(ncomp):
                    ob = psb.tile([PT, R], f32, name="ob", tag="ob")
                    for tw in range(NT):
                        nc.tensor.matmul(
                            ob[:],
                            m1t[:, c, tw, :],
                            ry_t[:, c, tw, :],
                            start=(tw == 0),
                            stop=(tw == NT - 1),
                        )
                    nc.scalar.copy(m2[:, c, :], ob[:])

            # ---- C(p-2): M3 = M2^T @ Lx^T ----
            for s in ([p - 2] if 0 <= p - 2 < SLICES else []):
                m2 = m2_tiles[s]
                m3 = m3_pool.tile([PT, ncomp, H], bf16, name=f"m3_{s}", tag="m3")
                m3_tiles[s] = m3
                for c in range(ncomp):
                    oc = psc.tile([PT, H], f32, name="oc", tag="oc")
                    nc.tensor.matmul(
                        oc[:], m2[:, c, :], lxt_t[:, c, :], start=True, stop=True
                    )
                    nc.scalar.copy(m3[:, c, :], oc[:])

            # Pulled-forward D for the final two slices (C just emitted).
            if p == NPH - 2:
                emit_d(SLICES - 2)
            if p == NPH - 1:
                emit_d(SLICES - 1)

    nc.compile()
    return nc


def _band(taps: np.ndarray, n: int) -> np.ndarray:
    """M[a, b] = taps[a - b + PAD] for |a - b| <= PAD, else 0."""
    M = np.zeros((n, n), np.float64)
    idx = np.arange(n)
    for d in range(-PAD, PAD + 1):
        b = idx[(idx + d >= 0) & (idx + d < n)]
        M[b + d, b] = taps[d + PAD]
    return M


def _factor(taps: np.ndarray, n: int):
    """Rank-R factorization L @ Rt of the 1D-conv operator T = band(taps)^T.

    T[i, k] = taps[k - i + PAD]: out[i] = sum_k T[i, k] x[k] is the
    zero-padded cross-correlation the reference computes.
    """
    T = _band(taps, n).T
    U, S, Vt = np.linalg.svd(T)
    L = U[:, :R] * np.sqrt(S[:R])
    Rt = Vt[:R].T * np.sqrt(S[:R])
    return L, Rt  # T ~= L @ Rt.T


def kernel(x: np.ndarray, kernel: np.ndarray) -> np.ndarray:
    x = np.asarray(x, dtype=np.float32)
    k2d = np.asarray(kernel, dtype=np.float32)
    n, c, h, w = x.shape
    assert (h, w) == (H, W) and k2d.shape == (KS, KS)

    # Separable decomposition of the 2D kernel (rank-1 for a Gaussian).
    U, S, Vt = np.linalg.svd(k2d.astype(np.float64))
    ncomp = max(1, int(np.sum(S > S[0] * 1e-7)))
    ncomp = min(ncomp, 4)

    kxs = [S[i] * U[:, i] for i in range(ncomp)]
    kys = [Vt[i] for i in range(ncomp)]
    sym = all(np.allclose(kxs[i], kys[i], atol=1e-9) for i in range(ncomp))

    rx = np.empty((ncomp, H, R), np.float32)
    ry = np.empty((ncomp, W, R), np.float32)
    lxt = np.empty((ncomp, R, H), np.float32)
    lyt = np.empty((ncomp, R, W), np.float32)
    for i in range(ncomp):
        Lx, Rx = _factor(kxs[i], H)
        rx[i] = Rx
        lxt[i] = Lx.T
        if sym:
            ry[i] = Rx
            lyt[i] = Lx.T
        else:
            Ly, Ry = _factor(kys[i], W)
            ry[i] = Ry
            lyt[i] = Ly.T

    # Device layouts: rx/ry -> [p, c, th, j]; lxt/lyt -> [p, c, j].
    rx_l = np.ascontiguousarray(
        rx.reshape(ncomp, NT, PT, R).transpose(2, 0, 1, 3)).astype(BF16)
    lxt_l = np.ascontiguousarray(lxt.transpose(1, 0, 2)).astype(BF16)

    key = (ncomp, sym)
    if key not in _kernel_cache:
        _kernel_cache[key] = _build_bass(ncomp, sym)
    nc = _kernel_cache[key]

    # x -> partition-major [p, s, th, j] relayout (slice ranges are single
    # per-partition-contiguous DMAs).
    xr = x.reshape(n * c, NT, PT, W).transpose(2, 0, 1, 3)  # [p, 48, th, j]
    xr = np.ascontiguousarray(xr).astype(BF16)
    per = xr.shape[1] // N_CORES
    consts = {"rx": rx_l, "lxt": lxt_l}
    if not sym:
        consts["ry"] = np.ascontiguousarray(
            ry.reshape(ncomp, NT, PT, R).transpose(2, 0, 1, 3)).astype(BF16)
        consts["lyt"] = np.ascontiguousarray(lyt.transpose(1, 0, 2)).astype(BF16)
    in_maps = [
        {"x": np.ascontiguousarray(xr[:, ci * per : (ci + 1) * per]), **consts}
        for ci in range(N_CORES)
    ]
    res = run_bass_kernel_spmd(nc, in_maps, core_ids=list(range(N_CORES)))
    global last_results
    last_results = res
    yl = np.concatenate(
        [res.results[ci]["y"] for ci in range(N_CORES)], axis=1
    )  # [p, 48, th, j]
    # Undo the partition-major relayout.
    y = yl.astype(np.float32).transpose(1, 2, 0, 3).reshape(n, c, h, w)
    return np.ascontiguousarray(y)


last_results = None
